# revision 12
# baseline (speedup 1.0000x reference)
"""Trainium2 Bass kernel for MuellerMatrixPyramid.

Input  x: [8, 48, 512, 512] f32  (16 I + 16 A + 16 W channel planes per batch)
Output:   [8, 34, 512, 512] f32

Math per pixel: M = inv(A) @ I @ inv(W), normalized by M[0,0].
Since M/M00 == (adj(A) @ I @ adj(W)) / (adj(A) @ I @ adj(W))[0,0] (the two
determinants cancel), we only need 4x4 adjugates (pure elementwise math),
two 4x4 matmuls (elementwise), one reciprocal and 16 scales.

Level 0 runs at 512x512; level 1 runs on the 2x2-maxpooled input at 256x256
and is bilinearly upsampled (align_corners) back to 512x512 via two
TensorEngine matmuls against a precomputed [256, 512] interpolation matrix.

Sharding: pure data parallelism, one batch element per NeuronCore (8 cores).
"""

import numpy as np

import concourse.bacc as bacc
import concourse.bass as bass
import concourse.tile as tile
from concourse import mybir
from concourse.bass_utils import run_bass_kernel_spmd

F32 = mybir.dt.float32
MULT = mybir.AluOpType.mult
ADD = mybir.AluOpType.add
SUB = mybir.AluOpType.subtract
MAX = mybir.AluOpType.max

B, NCH, H, W = 8, 48, 512, 512
HW = H * W          # 262144, flat plane = [128, 2048]
P = 128
F0 = 512            # level-0 free-dim chunk (plane = 4 chunks)
NCHUNK0 = HW // (P * F0)
H1, W1 = 256, 256
HW1 = H1 * W1       # 65536, flat = [128, 512]
F1 = 512

# ---------------------------------------------------------------------------
# adjugate formulas (2x2 cofactor method), validated vs numpy.
# each entry is (m1 - m2) op m3 with products m_i = a[r1,c1]*sc[k]
# entry spec: (i, j, (r,c,k)x3, last_op) meaning
#   b[i][j] = (a[r0,c0]*t[k0] - a[r1,c1]*t[k1]) op a[r2,c2]*t[k2]
# where t = [s0..s5, c0..c5] index 0..11
S0, S1, S2, S3, S4, S5, C0, C1, C2, C3, C4, C5 = range(12)

SC_SPEC = [
    # (out_idx, (r,c) of first factor, (r,c) of second, (r,c) third, (r,c) fourth)
    # t[k] = a[ra,ca]*a[rb,cb] - a[rc,cc]*a[rd,cd]
    (S0, (0, 0), (1, 1), (1, 0), (0, 1)),
    (S1, (0, 0), (1, 2), (1, 0), (0, 2)),
    (S2, (0, 0), (1, 3), (1, 0), (0, 3)),
    (S3, (0, 1), (1, 2), (1, 1), (0, 2)),
    (S4, (0, 1), (1, 3), (1, 1), (0, 3)),
    (S5, (0, 2), (1, 3), (1, 2), (0, 3)),
    (C5, (2, 2), (3, 3), (3, 2), (2, 3)),
    (C4, (2, 1), (3, 3), (3, 1), (2, 3)),
    (C3, (2, 1), (3, 2), (3, 1), (2, 2)),
    (C2, (2, 0), (3, 3), (3, 0), (2, 3)),
    (C1, (2, 0), (3, 2), (3, 0), (2, 2)),
    (C0, (2, 0), (3, 1), (3, 0), (2, 1)),
]

ADJ_SPEC = [
    # (i, j, (r,c,k), (r,c,k), (r,c,k), last_op)
    # b[i][j] = (a0*t0 - a1*t1) last_op a2*t2
    (0, 0, (1, 1, C5), (1, 2, C4), (1, 3, C3), ADD),
    (0, 1, (0, 2, C4), (0, 1, C5), (0, 3, C3), SUB),
    (0, 2, (3, 1, S5), (3, 2, S4), (3, 3, S3), ADD),
    (0, 3, (2, 2, S4), (2, 1, S5), (2, 3, S3), SUB),
    (1, 0, (1, 2, C2), (1, 0, C5), (1, 3, C1), SUB),
    (1, 1, (0, 0, C5), (0, 2, C2), (0, 3, C1), ADD),
    (1, 2, (3, 2, S2), (3, 0, S5), (3, 3, S1), SUB),
    (1, 3, (2, 0, S5), (2, 2, S2), (2, 3, S1), ADD),
    (2, 0, (1, 0, C4), (1, 1, C2), (1, 3, C0), ADD),
    (2, 1, (0, 1, C2), (0, 0, C4), (0, 3, C0), SUB),
    (2, 2, (3, 0, S4), (3, 1, S2), (3, 3, S0), ADD),
    (2, 3, (2, 1, S2), (2, 0, S4), (2, 3, S0), SUB),
    (3, 0, (1, 1, C1), (1, 0, C3), (1, 2, C0), SUB),
    (3, 1, (0, 0, C3), (0, 1, C1), (0, 2, C0), ADD),
    (3, 2, (3, 1, S1), (3, 0, S3), (3, 2, S0), SUB),
    (3, 3, (2, 0, S3), (2, 1, S1), (2, 2, S0), ADD),
]


def up_matrix(h: int, oh: int) -> np.ndarray:
    """[oh, h] bilinear align_corners upsample matrix (2 nnz per row)."""
    ys = np.linspace(0.0, h - 1.0, oh, dtype=np.float32)
    y0 = np.floor(ys).astype(np.int32)
    y1 = np.minimum(y0 + 1, h - 1)
    fy = (ys - y0).astype(np.float32)
    U = np.zeros((oh, h), dtype=np.float32)
    np.add.at(U, (np.arange(oh), y0), 1.0 - fy)
    np.add.at(U, (np.arange(oh), y1), fy)
    return U


class EngineMix:
    """Round-robin split of elementwise work between engines by weight."""

    def __init__(self, nc, gps_share=0.30):
        self.nc = nc
        self.gps_share = gps_share
        self.acc = 0.0

    def pick(self):
        self.acc += self.gps_share
        if self.acc >= 1.0:
            self.acc -= 1.0
            return self.nc.gpsimd
        return self.nc.vector


def _emit_adjugate(mix, sc_tiles, adj, xt, F):
    """Emit s/c terms + 16 adjugate entry planes.

    xt:  [128, 16, F] input matrix tile (entry (r,c) at plane 4r+c)
    adj: [128, 16, F] output tile (entry (i,j) at plane 4i+j)
    sc_tiles: 6 [128, F] scratch tiles shared by the c-batch then the s-batch
    """

    def a(r, c):
        return xt[:, 4 * r + c, :]

    def emit_sc(batch):
        # t = a0*a1 - a2*a3  (3 ops each); slot = k % 6
        for (k, p0, p1, p2, p3) in SC_SPEC:
            if (k >= 6) != batch_is_c:
                continue
            t = sc_tiles[k % 6]
            mix.pick().tensor_tensor(t, a(*p0), a(*p1), MULT)
            m2 = mix.scratch()
            mix.pick().tensor_tensor(m2, a(*p2), a(*p3), MULT)
            mix.pick().tensor_tensor(t, t, m2, SUB)

    def emit_entries():
        # b = (a0*t0 - a1*t1) op a2*t2  (5 ops each)
        for (i, j, f0, f1, f2, op) in ADJ_SPEC:
            uses_c = f0[2] >= 6
            if uses_c != batch_is_c:
                continue
            o = adj[:, 4 * i + j, :]
            m1 = mix.scratch()
            m2 = mix.scratch()
            mix.pick().tensor_tensor(m1, a(f0[0], f0[1]), sc_tiles[f0[2] % 6], MULT)
            mix.pick().tensor_tensor(m2, a(f1[0], f1[1]), sc_tiles[f1[2] % 6], MULT)
            mix.pick().tensor_tensor(m1, m1, m2, SUB)
            mix.pick().tensor_tensor(m2, a(f2[0], f2[1]), sc_tiles[f2[2] % 6], MULT)
            mix.pick().tensor_tensor(o, m1, m2, op)

    for batch_is_c in (True, False):
        emit_sc(batch_is_c)
        emit_entries()


def _emit_mm4(mix, out, lhs, rhs, scratch):
    """out[i,j] = sum_k lhs[i,k] * rhs[k,j] ; all tiles [128, 16, F]."""
    for i in range(4):
        for j in range(4):
            o = out[:, 4 * i + j, :]
            m = scratch()
            mix.pick().tensor_tensor(o, lhs[:, 4 * i + 0, :], rhs[:, 0 + j, :], MULT)
            for k in range(1, 4):
                mix.pick().tensor_tensor(m, lhs[:, 4 * i + k, :], rhs[:, 4 * k + j, :], MULT)
                mix.pick().tensor_tensor(o, o, m, ADD)


def build_nc():
    nc = bacc.Bacc("TRN2", target_bir_lowering=False, debug=False, num_devices=B)

    x = nc.declare_dram_parameter("x", [NCH, H, W], F32, isOutput=False)
    ut = nc.declare_dram_parameter("ut", [H1, H], F32, isOutput=False)  # U^T [256,512]
    out = nc.declare_dram_parameter("out", [34, H, W], F32, isOutput=True)
    pooled = nc.dram_tensor("pooled", [NCH, H1, W1], F32)

    x_flat = x[:].rearrange("c h w -> c (h w)")           # [48, 262144]
    out_flat = out[:].rearrange("c h w -> c (h w)")       # [34, 262144]
    pooled_flat = pooled[:].rearrange("c h w -> c (h w)")  # [48, 65536]

    with tile.TileContext(nc) as tc:
        import contextlib
        ctx = contextlib.ExitStack()
        with ctx:
            xgrp = ctx.enter_context(tc.tile_pool(name="xgrp", bufs=2))
            scp = ctx.enter_context(tc.tile_pool(name="scp", bufs=1))
            adjp = ctx.enter_context(tc.tile_pool(name="adjp", bufs=1))
            n1p = ctx.enter_context(tc.tile_pool(name="n1p", bufs=1))
            tmpp = ctx.enter_context(tc.tile_pool(name="tmpp", bufs=2))
            outp = ctx.enter_context(tc.tile_pool(name="outp", bufs=2))
            poolp = ctx.enter_context(tc.tile_pool(name="poolp", bufs=2))
            upp = ctx.enter_context(tc.tile_pool(name="upp", bufs=2))
            singles = ctx.enter_context(tc.tile_pool(name="singles", bufs=1))
            psum1 = ctx.enter_context(tc.tile_pool(name="psum1", bufs=2, space="PSUM"))
            psum2 = ctx.enter_context(tc.tile_pool(name="psum2", bufs=4, space="PSUM"))

            # upsample matrix, resident: [128, 2, 512]; partition p holds
            # U^T rows p (half 0) and 128+p (half 1)
            ut_sb = singles.tile([P, 2, H], F32)
            nc.sync.dma_start(
                out=ut_sb, in_=ut[:].rearrange("(h p) o -> p h o", p=P)
            )

            mix = EngineMix(nc, gps_share=0.38)

            def make_scratch(pool, F):
                state = {"i": 0}

                def scratch():
                    state["i"] += 1
                    return pool.tile([P, F], F32, tag=f"scr{state['i'] % 8}", name=f"scr{state['i'] % 8}")

                return scratch

            # ------------------------------------------------------------------
            # maxpool pass (input 48ch 512x512 -> pooled 256x256), on gpsimd
            # chunk half h of channel c: [128, 1024] = image rows (4p+2h, 4p+2h+1)
            def emit_pool():
                xv = x_flat.rearrange("c (p f) -> c p f", p=P)  # [48,128,2048]
                pv = pooled_flat.rearrange("c (p f) -> c p f", p=P)  # [48,128,512]
                for c in range(NCH):
                    for h in range(2):
                        xt = poolp.tile([P, 1024], F32, tag="poolx", name="poolx")
                        nc.sync.dma_start(out=xt, in_=xv[c, :, h * 1024:(h + 1) * 1024])
                        rm = poolp.tile([P, 512], F32, tag="poolr", name="poolr")
                        nc.vector.tensor_tensor(rm, xt[:, 0:512], xt[:, 512:1024], MAX)
                        pl = poolp.tile([P, 256], F32, tag="poolo", name="poolo")
                        nc.vector.tensor_tensor(pl, rm[:, 0:512:2], rm[:, 1:512:2], MAX)
                        nc.sync.dma_start(
                            out=pv[c, :, h * 256:(h + 1) * 256], in_=pl
                        )

            # ------------------------------------------------------------------
            # feature pipeline for one pixel chunk.
            # load_grp(g, F) -> [128,16,F] tile APs for group g in {0:I,1:A,2:W}
            def emit_features(load_grp, store_out, F, scratch):
                """Computes the 17 feature planes; returns (inten, n1) where
                inten [P,F] is the scaled intensity and n1 [P,16,F] holds the
                normalized Mueller planes (entry (i,j) at plane 4i+j)."""
                mix.scratch = scratch
                sc_tiles = [scp.tile([P, F], F32, tag=f"sc{k}", name=f"sc{k}")
                            for k in range(6)]

                # A -> adjA
                at = load_grp(1)
                adja = adjp.tile([P, 16, F], F32, tag="adj", name="adja")
                _emit_adjugate(mix, sc_tiles, adja, at, F)

                # I: intensity + N1 = adjA @ I
                it = load_grp(0)
                inten = outp.tile([P, F], F32, tag="inten", name="inten")
                nc.vector.tensor_reduce(
                    inten, it.rearrange("p k f -> p f k"),
                    axis=mybir.AxisListType.X, op=ADD,
                )
                nc.scalar.mul(inten, inten, 1.0 / 16.0)
                n1 = n1p.tile([P, 16, F], F32, tag="n1", name="n1")
                _emit_mm4(mix, n1, adja, it, scratch)

                # W -> adjW
                wt = load_grp(2)
                adjw = adjp.tile([P, 16, F], F32, tag="adj", name="adjw")
                _emit_adjugate(mix, sc_tiles, adjw, wt, F)

                # N = N1 @ adjW row by row into scratch, then write the
                # normalized value back over the n1 row (in-place).
                r = outp.tile([P, F], F32, tag="recip", name="recip")
                for i in range(4):
                    row = []
                    for j in range(4):
                        o = scratch()
                        m = scratch()
                        mix.pick().tensor_tensor(o, n1[:, 4 * i, :], adjw[:, j, :], MULT)
                        for k in range(1, 4):
                            mix.pick().tensor_tensor(
                                m, n1[:, 4 * i + k, :], adjw[:, 4 * k + j, :], MULT)
                            mix.pick().tensor_tensor(o, o, m, ADD)
                        row.append(o)
                    if i == 0:
                        nc.vector.reciprocal(r, row[0])
                    for j in range(4):
                        mix.pick().tensor_tensor(n1[:, 4 * i + j, :], row[j], r, MULT)

                store_out(inten, n1)
                return inten, n1

            # ------------------------------------------------------------------
            # level 0: 4 chunks of [128, 512] over the flat plane
            scratch0 = make_scratch(tmpp, F0)
            for ch in range(NCHUNK0):
                fsl = slice(ch * F0, (ch + 1) * F0)

                def load_grp(g, _fsl=fsl):
                    t = xgrp.tile([P, 16, F0], F32, tag="xg", name="xg")
                    src = x_flat[16 * g:16 * (g + 1), :].rearrange(
                        "c (p f) -> p c f", p=P
                    )[:, :, _fsl]
                    nc.sync.dma_start(out=t, in_=src)
                    return t

                def store_out(inten, n1, _fsl=fsl):
                    ov = out_flat.rearrange("c (p f) -> p c f", p=P)
                    nc.sync.dma_start(out=ov[:, 0, _fsl], in_=inten)
                    nc.sync.dma_start(out=ov[:, 1:17, _fsl], in_=n1)

                emit_features(load_grp, store_out, F0, scratch0)

            # pooling pass (emitted after level-0 so level-0 output DMAs get
            # priority early; engines interleave anyway)
            emit_pool()

            # ------------------------------------------------------------------
            # level 1: one chunk [128, 512]; partition p = image rows p & 128+p
            # (layout [128, 2, 256]: half h = row h*128+p, cols 0..255)
            scratch1 = make_scratch(tmpp, F1)

            def load_grp1(g):
                t = xgrp.tile([P, 16, F1], F32, tag="xg", name="xg1")
                src = pooled_flat[16 * g:16 * (g + 1), :].rearrange(
                    "c (h p w) -> p c h w", p=P, h=2
                )
                nc.sync.dma_start(out=t.rearrange("p c (h w) -> p c h w", h=2), in_=src)
                return t

            inten1, n1_l1 = emit_features(load_grp1, lambda i, n: None, F1, scratch1)

            # upsample each plane: out = U @ m @ U^T via two PE matmul stages
            # plane layout [128, 2, 256]: half h = image row h*128+p, cols 0..255
            for c in range(17):
                src_pl = inten1 if c == 0 else n1_l1[:, c - 1, :]
                m = src_pl.rearrange("p (h w) -> p h w", h=2)  # [128,2,256]
                # stage 1: tAT[w, oy] = sum_y m[y, w] * UT[y, oy]
                tat = upp.tile([P, 2, H], F32, tag="tat", name="tat", bufs=1)  # [128, 2, 512], w=q*128+p
                for q in range(2):
                    ps = psum1.tile([P, H], F32, tag="ps1", name="ps1")
                    for hh in range(2):
                        nc.tensor.matmul(
                            ps,
                            m[:, hh, q * P:(q + 1) * P],   # lhsT [y(128), w(128)]
                            ut_sb[:, hh, :],               # rhs [y(128), oy(512)]
                            start=(hh == 0), stop=(hh == 1),
                        )
                    nc.scalar.copy(tat[:, q, :], ps)
                # stage 2: out[oy, ox] = sum_w tat[w, oy] * UT[w, ox]
                for oc in range(4):
                    ps = psum2.tile([P, W], F32, tag="ps2", name="ps2")
                    for q in range(2):
                        nc.tensor.matmul(
                            ps,
                            tat[:, q, oc * P:(oc + 1) * P],  # lhsT [w(128), oy(128)]
                            ut_sb[:, q, :],                  # rhs [w(128), ox(512)]
                            start=(q == 0), stop=(q == 1),
                        )
                    ob = upp.tile([P, W], F32, tag="upout", name="upout", bufs=2)
                    nc.scalar.copy(ob, ps)
                    dst = out[17 + c, oc * P:(oc + 1) * P, :]
                    nc.sync.dma_start(out=dst, in_=ob)

    nc.compile()
    return nc


_NC = None


def _get_nc():
    global _NC
    if _NC is None:
        _NC = build_nc()
    return _NC


def kernel(x: np.ndarray) -> np.ndarray:
    assert x.shape == (B, NCH, H, W), x.shape
    nc = _get_nc()
    utm = np.ascontiguousarray(up_matrix(H1, H).T)  # [256, 512]
    in_maps = [
        {"x": np.ascontiguousarray(x[b], dtype=np.float32), "ut": utm}
        for b in range(B)
    ]
    res = run_bass_kernel_spmd(nc, in_maps, list(range(B))).results
    out = np.stack([res[b]["out"] for b in range(B)], axis=0)
    return out.astype(np.float32)
